# revision 20
# baseline (speedup 1.0000x reference)
"""CrossEntropyLossWithGaussianSmoothedLabels on 8 TRN2 NeuronCores.

Math: the reference's scatter-built smoothed label at class j is w[|j-t|]
for |j-t|<=3 (w = [1, e^-.5, e^-1, e^-2]); clamped edge writes are always
overwritten by the nearer-distance write. So

  loss = mean_r( W_r * logsumexp(x_r) - sum_o w[|o|] * x_r[t_r+o] )

Device computes the two O(rows*classes) reductions; the O(rows)
postprocessing (ln, W_r weighting, final mean) runs on the host in
float64:

  - per-row sumexp: ACT streams exp over the fp8 logits in a few large
    macro instructions (one table load, no per-tile accumulator reads);
    DVE folds+sums each 128-row tile with one fused
    scalar_tensor_tensor+accum_out over the f16 exp halves.
  - gather term: 6 banded 128x128 fp8 matmuls per tile accumulate the
    H^T X class-block diagonals in PSUM across all 32 tiles; one fused
    scalar_tensor_tensor+accum contracts PSUM against the band-weight
    masks at the end.

The one-hot H is built host-side and shipped interleaved with x as fp8
(total upload = 2 * 2.96 MB/core, same bytes as f16 x alone), which
removes the per-tile is_eq H-build from the Vector engine entirely.
DMA chunk completions are aggregated in order by the otherwise-idle
GPSIMD sequencer so consumers wait one cumulative semaphore despite
possible out-of-order same-queue DMA completion.
"""

import math

import numpy as np
import ml_dtypes

import concourse.bacc as bacc
from concourse import mybir
from concourse.bass_utils import run_bass_kernel_spmd

P = 128
C = 722
H1 = 361                # fold split for the rowsum
NCORES = 8
ROWS = 16 * 2048
RPC = ROWS // NCORES    # 4096 rows per core
NT = RPC // P           # 32 row-tiles per core
NB = 6
BLK = [0, 124, 248, 372, 496, 594]
URANGES = [(0, 124), (124, 248), (248, 372), (372, 496), (496, 594), (594, 722)]
WDEC = [1.0, math.exp(-0.5), math.exp(-1.0), math.exp(-2.0)]

CHUNKS = [2, 2, 4, 4, 4, 4, 4, 4, 4]         # DMA chunk sizes in tiles
EXPS = [2, 2, 4, 8, 8, 4, 2, 1, 1]           # ACT exp macro sizes
OUT1 = 24                                     # sumexp cols shipped early

f8 = mybir.dt.float8e4
f16 = mybir.dt.float16
f32 = mybir.dt.float32
npf8 = ml_dtypes.float8_e4m3

assert sum(CHUNKS) == NT and sum(EXPS) == NT


def _band_masks() -> np.ndarray:
    """[128, 6*128] f32: block-local band weights, each global band entry
    owned by exactly one block (by min(m,n) ownership range)."""
    m = np.zeros((P, NB * P), np.float32)
    for b in range(NB):
        s = BLK[b]
        lo, hi = URANGES[b]
        for i in range(P):
            for o in range(-3, 4):
                j = i + o
                if 0 <= j < P:
                    mg, ng = s + i, s + j
                    if mg < C and ng < C and lo <= min(mg, ng) < hi:
                        m[i, b * P + j] = WDEC[abs(o)]
    return m


def _build():
    nc = bacc.Bacc(
        "TRN2", target_bir_lowering=False, debug=False, num_devices=NCORES
    )
    AF = mybir.ActivationFunctionType
    OP = mybir.AluOpType

    # x and one-hot H interleaved per tile: [:, i, 0, :] = x, [:, i, 1, :] = H
    xhd = nc.dram_tensor("xh", [P, NT, 2, C], f8, kind="ExternalInput").ap()
    bandd = nc.dram_tensor("band", [P, NB * P], f32, kind="ExternalInput").ap()
    outd = nc.dram_tensor("out", [P, NT + 1], f32, kind="ExternalOutput").ap()

    xh_all = nc.alloc_sbuf_tensor("xh_all", [P, NT, 2, C], f8).ap()
    esc_all = nc.alloc_sbuf_tensor("esc_all", [P, NT, C], f16).ap()
    band_sb = nc.alloc_sbuf_tensor("band_sb", [P, NB * P], f32).ap()
    scr = nc.alloc_sbuf_tensor("scr", [P, H1], f16).ap()
    scrb = nc.alloc_sbuf_tensor("scrb", [P, NB * P], f32).ap()
    outsb = nc.alloc_sbuf_tensor("outsb", [P, NT + 1], f32).ap()
    warm = nc.alloc_sbuf_tensor("warm", [P, 1], f32).ap()
    warmo = nc.alloc_sbuf_tensor("warmo", [P, 1], f32).ap()

    psum = nc.alloc_psum_tensor("psumblk", [P, NB, 512], f32).ap()

    ch_end = np.cumsum(CHUNKS).tolist()          # chunk -> first tile after
    ch_start = [e - s for e, s in zip(ch_end, CHUNKS)]

    def chunk_of(tile):                           # chunks needed through tile
        for c, e in enumerate(ch_end):
            if tile < e:
                return c + 1
        raise AssertionError

    spans = []
    a = 0
    for s in EXPS:
        spans.append((a, a + s))
        a += s

    with (
        nc.Block() as block,
        nc.semaphore("aux") as aux,
        nc.semaphore("act_sem") as act_sem,
        nc.semaphore("pe_tile") as pe_tile,
        nc.semaphore("vsum") as vsum,
        nc.semaphore("vfin") as vfin,
        nc.semaphore("osem") as osem,
        nc.semaphore("xs0") as xs0,
        nc.semaphore("xs1") as xs1,
        nc.semaphore("xs2") as xs2,
        nc.semaphore("xs3") as xs3,
        nc.semaphore("xs4") as xs4,
        nc.semaphore("xs5") as xs5,
        nc.semaphore("xs6") as xs6,
        nc.semaphore("xs7") as xs7,
        nc.semaphore("xs8") as xs8,
    ):
        xs = [xs0, xs1, xs2, xs3, xs4, xs5, xs6, xs7, xs8]

        # flat per-partition-contiguous views so each chunk lowers to one
        # large descriptor per partition (full DMA bandwidth)
        xh_flat = xh_all.rearrange("p t two c -> p (t two c)")
        xhd_flat = xhd.rearrange("p t two c -> p (t two c)")
        TW = 2 * C

        SP_CHUNKS = (0, 1)    # first chunks on the near-empty SP queue so
                              # their completion sems fire promptly; the rest
                              # stream via GPSIMD's software DGE queue

        def wait_chunks(eng, t0, t1):
            """wait for every chunk overlapping tiles [t0, t1)"""
            for c in range(len(CHUNKS)):
                if ch_end[c] > t0 and ch_start[c] < t1:
                    eng.wait_ge(xs[c], 16)

        @block.sync
        def _(sync):
            for c in SP_CHUNKS:
                sync.dma_start(
                    out=xh_flat[:, ch_start[c] * TW:ch_end[c] * TW],
                    in_=xhd_flat[:, ch_start[c] * TW:ch_end[c] * TW],
                ).then_inc(xs[c], 16)
            sync.dma_start(out=band_sb, in_=bandd).then_inc(aux, 16)
            sync.wait_ge(vsum, 3)
            sync.dma_start(out=outd[:, 0:OUT1], in_=outsb[:, 0:OUT1]).then_inc(
                osem, 16)
            sync.wait_ge(vfin, 1)
            sync.dma_start(
                out=outd[:, OUT1:NT + 1], in_=outsb[:, OUT1:NT + 1]
            ).then_inc(osem, 16)
            sync.wait_ge(osem, 32)

        @block.gpsimd
        def _(gpsimd):
            for c in range(len(SP_CHUNKS), len(CHUNKS)):
                gpsimd.dma_start(
                    out=xh_flat[:, ch_start[c] * TW:ch_end[c] * TW],
                    in_=xhd_flat[:, ch_start[c] * TW:ch_end[c] * TW],
                ).then_inc(xs[c], 16)

        @block.scalar
        def _(scalar):
            # load the Exp table set during the DMA fill
            scalar.memzero(warm)
            scalar.activation(out=warmo, in_=warm, func=AF.Exp)
            for (a0, a1) in spans:
                wait_chunks(scalar, a0, a1)
                scalar.activation(
                    out=esc_all[:, a0:a1, :], in_=xh_all[:, a0:a1, 0, :],
                    func=AF.Exp,
                ).then_inc(act_sem, a1 - a0)

        @block.vector
        def _(vector):
            def rowsum(j):
                vector.wait_ge(act_sem, j + 1)
                ins = vector.scalar_tensor_tensor(
                    out=scr, in0=esc_all[:, j, 0:H1], scalar=1.0,
                    in1=esc_all[:, j, H1:C], op0=OP.mult, op1=OP.add,
                    accum_out=outsb[:, j:j + 1],
                )
                if j % 8 == 7:
                    ins.then_inc(vsum, 1)
                return ins

            for j in range(OUT1):
                rowsum(j)
            # band extraction early: PE is long done by now, and this keeps
            # it off the post-ACT critical path
            vector.wait_ge(pe_tile, NT)
            vector.wait_ge(aux, 16)
            vector.scalar_tensor_tensor(
                out=scrb.rearrange("p (b n) -> p b n", b=NB),
                in0=psum[:, :, 0:P], scalar=1.0,
                in1=band_sb.rearrange("p (b n) -> p b n", b=NB),
                op0=OP.mult, op1=OP.mult,
                accum_out=outsb[:, NT:NT + 1],
            )
            for j in range(OUT1, NT):
                rowsum(j)
            vector.engine_nop().then_inc(vfin, 1)

        @block.tensor
        def _(pe):
            for i in range(NT):
                wait_chunks(pe, i, i + 1)
                for b in range(NB):
                    s0 = BLK[b]
                    mm = pe.matmul(
                        psum[:, b, 0:P], xh_all[:, i, 1, s0:s0 + P],
                        xh_all[:, i, 0, s0:s0 + P],
                        start=(i == 0), stop=(i == NT - 1),
                    )
                mm.then_inc(pe_tile, 1)

    nc.compile()
    return nc


def _shard_inputs(prediction: np.ndarray, target: np.ndarray):
    pred = np.asarray(prediction, dtype=np.float32).reshape(-1, C)
    tgt = np.asarray(target).reshape(-1).astype(np.int64)
    x8 = pred.astype(npf8)
    h8 = np.zeros((ROWS, C), npf8)
    h8[np.arange(ROWS), tgt] = 1.0
    band = _band_masks()
    in_maps = []
    for c in range(NCORES):
        sl = slice(c * RPC, (c + 1) * RPC)
        xh = np.empty((P, NT, 2, C), npf8)
        xh[:, :, 0, :] = x8[sl].reshape(NT, P, C).transpose(1, 0, 2)
        xh[:, :, 1, :] = h8[sl].reshape(NT, P, C).transpose(1, 0, 2)
        in_maps.append({"xh": xh, "band": band})
    return in_maps, tgt


def _host_combine(results, tgt: np.ndarray) -> np.float32:
    w1, w2, w3 = WDEC[1], WDEC[2], WDEC[3]
    t = tgt
    W = (1.0
         + w1 * ((t >= 1).astype(np.float64) + (t <= C - 2))
         + w2 * ((t >= 2).astype(np.float64) + (t <= C - 3))
         + w3 * ((t >= 3).astype(np.float64) + (t <= C - 4)))
    tot = 0.0
    for c, r in enumerate(results):
        o = np.asarray(r["out"], dtype=np.float64)
        S = o[:, 0:NT]                    # S[p, i] = sumexp of row i*P+p
        lse = np.log(S)
        Wc = W[c * RPC:(c + 1) * RPC].reshape(NT, P).T
        tot += (Wc * lse).sum() - o[:, NT].sum()
    return np.float32(tot / ROWS)


def kernel(prediction: np.ndarray, target: np.ndarray, _trace: bool = False):
    nc = _build()
    in_maps, tgt = _shard_inputs(prediction, target)
    res = run_bass_kernel_spmd(
        nc, in_maps, core_ids=list(range(NCORES)), trace=_trace
    )
    loss = _host_combine(res.results, tgt)
    if _trace:
        return loss, res
    return loss


# revision 21
# speedup vs baseline: 1.0569x; 1.0569x over previous
"""CrossEntropyLossWithGaussianSmoothedLabels on 8 TRN2 NeuronCores.

Math: the reference's scatter-built smoothed label at class j is w[|j-t|]
for |j-t|<=3 (w = [1, e^-.5, e^-1, e^-2]); clamped edge writes are always
overwritten by the nearer-distance write. So

  loss = mean_r( W_r * logsumexp(x_r) - sum_o w[|o|] * x_r[t_r+o] )

Device computes the two O(rows*classes) reductions; the O(rows)
postprocessing (ln, W_r weighting, final mean) runs on the host in
float64:

  - per-row sumexp: ACT streams exp over the fp8 logits in a few large
    macro instructions (one table load, no per-tile accumulator reads);
    DVE folds+sums each 128-row tile with one fused
    scalar_tensor_tensor+accum_out over the f16 exp halves.
  - gather term: 6 banded 128x128 fp8 matmuls per tile accumulate the
    H^T X class-block diagonals in PSUM across all 32 tiles; one fused
    scalar_tensor_tensor+accum contracts PSUM against the band-weight
    masks at the end.

The one-hot H is built host-side and shipped interleaved with x as fp8
(total upload = 2 * 2.96 MB/core, same bytes as f16 x alone), which
removes the per-tile is_eq H-build from the Vector engine entirely.
DMA chunk completions are aggregated in order by the otherwise-idle
GPSIMD sequencer so consumers wait one cumulative semaphore despite
possible out-of-order same-queue DMA completion.
"""

import math

import numpy as np
import ml_dtypes

import concourse.bacc as bacc
from concourse import mybir
from concourse.bass_utils import run_bass_kernel_spmd

P = 128
C = 722
H1 = 361                # fold split for the rowsum
NCORES = 8
ROWS = 16 * 2048
RPC = ROWS // NCORES    # 4096 rows per core
NT = RPC // P           # 32 row-tiles per core
NB = 6
BLK = [0, 124, 248, 372, 496, 594]
URANGES = [(0, 124), (124, 248), (248, 372), (372, 496), (496, 594), (594, 722)]
WDEC = [1.0, math.exp(-0.5), math.exp(-1.0), math.exp(-2.0)]

CHUNKS = [2, 2, 4, 4, 4, 4, 4, 4, 4]         # DMA chunk sizes in tiles
EXPS = [2, 2, 4, 8, 4, 4, 4, 2, 1, 1]        # ACT exp macro sizes
OUT1 = 24                                     # sumexp cols shipped early

f8 = mybir.dt.float8e4
f16 = mybir.dt.float16
f32 = mybir.dt.float32
npf8 = ml_dtypes.float8_e4m3

assert sum(CHUNKS) == NT and sum(EXPS) == NT


def _band_masks() -> np.ndarray:
    """[128, 6*128] f32: block-local band weights, each global band entry
    owned by exactly one block (by min(m,n) ownership range)."""
    m = np.zeros((P, NB * P), np.float32)
    for b in range(NB):
        s = BLK[b]
        lo, hi = URANGES[b]
        for i in range(P):
            for o in range(-3, 4):
                j = i + o
                if 0 <= j < P:
                    mg, ng = s + i, s + j
                    if mg < C and ng < C and lo <= min(mg, ng) < hi:
                        m[i, b * P + j] = WDEC[abs(o)]
    return m


def _build():
    nc = bacc.Bacc(
        "TRN2", target_bir_lowering=False, debug=False, num_devices=NCORES
    )
    AF = mybir.ActivationFunctionType
    OP = mybir.AluOpType

    # x and one-hot H interleaved per tile: [:, i, 0, :] = x, [:, i, 1, :] = H
    xhd = nc.dram_tensor("xh", [P, NT, 2, C], f8, kind="ExternalInput").ap()
    bandd = nc.dram_tensor("band", [P, NB * P], f32, kind="ExternalInput").ap()
    outd = nc.dram_tensor("out", [P, NT + 1], f32, kind="ExternalOutput").ap()

    xh_all = nc.alloc_sbuf_tensor("xh_all", [P, NT, 2, C], f8).ap()
    esc_all = nc.alloc_sbuf_tensor("esc_all", [P, NT, C], f16).ap()
    band_sb = nc.alloc_sbuf_tensor("band_sb", [P, NB * P], f32).ap()
    scr = nc.alloc_sbuf_tensor("scr", [P, H1], f16).ap()
    scrb = nc.alloc_sbuf_tensor("scrb", [P, NB * P], f32).ap()
    outsb = nc.alloc_sbuf_tensor("outsb", [P, NT + 1], f32).ap()
    warm = nc.alloc_sbuf_tensor("warm", [P, 1], f32).ap()
    warmo = nc.alloc_sbuf_tensor("warmo", [P, 1], f32).ap()

    psum = nc.alloc_psum_tensor("psumblk", [P, NB, 512], f32).ap()

    ch_end = np.cumsum(CHUNKS).tolist()          # chunk -> first tile after
    ch_start = [e - s for e, s in zip(ch_end, CHUNKS)]

    def chunk_of(tile):                           # chunks needed through tile
        for c, e in enumerate(ch_end):
            if tile < e:
                return c + 1
        raise AssertionError

    spans = []
    a = 0
    for s in EXPS:
        spans.append((a, a + s))
        a += s

    with (
        nc.Block() as block,
        nc.semaphore("aux") as aux,
        nc.semaphore("act_sem") as act_sem,
        nc.semaphore("pe_tile") as pe_tile,
        nc.semaphore("vsum") as vsum,
        nc.semaphore("vfin") as vfin,
        nc.semaphore("osem") as osem,
        nc.semaphore("xs0") as xs0,
        nc.semaphore("xs1") as xs1,
        nc.semaphore("xs2") as xs2,
        nc.semaphore("xs3") as xs3,
        nc.semaphore("xs4") as xs4,
        nc.semaphore("xs5") as xs5,
        nc.semaphore("xs6") as xs6,
        nc.semaphore("xs7") as xs7,
        nc.semaphore("xs8") as xs8,
    ):
        xs = [xs0, xs1, xs2, xs3, xs4, xs5, xs6, xs7, xs8]

        # flat per-partition-contiguous views so each chunk lowers to one
        # large descriptor per partition (full DMA bandwidth)
        xh_flat = xh_all.rearrange("p t two c -> p (t two c)")
        xhd_flat = xhd.rearrange("p t two c -> p (t two c)")
        TW = 2 * C

        SP_CHUNKS = (0,)      # chunk0 at the head of the near-empty SP
                              # queue; the rest head the GPSIMD software-DGE
                              # queue so early sems fire promptly on both

        def wait_chunks(eng, t0, t1):
            """wait for every chunk overlapping tiles [t0, t1)"""
            for c in range(len(CHUNKS)):
                if ch_end[c] > t0 and ch_start[c] < t1:
                    eng.wait_ge(xs[c], 16)

        @block.sync
        def _(sync):
            for c in SP_CHUNKS:
                sync.dma_start(
                    out=xh_flat[:, ch_start[c] * TW:ch_end[c] * TW],
                    in_=xhd_flat[:, ch_start[c] * TW:ch_end[c] * TW],
                ).then_inc(xs[c], 16)
            sync.dma_start(out=band_sb, in_=bandd).then_inc(aux, 16)
            sync.wait_ge(vsum, 3)
            sync.dma_start(out=outd[:, 0:OUT1], in_=outsb[:, 0:OUT1]).then_inc(
                osem, 16)
            sync.wait_ge(vfin, 1)
            sync.dma_start(
                out=outd[:, OUT1:NT + 1], in_=outsb[:, OUT1:NT + 1]
            ).then_inc(osem, 16)
            sync.wait_ge(osem, 32)

        @block.gpsimd
        def _(gpsimd):
            for c in range(len(SP_CHUNKS), len(CHUNKS)):
                gpsimd.dma_start(
                    out=xh_flat[:, ch_start[c] * TW:ch_end[c] * TW],
                    in_=xhd_flat[:, ch_start[c] * TW:ch_end[c] * TW],
                ).then_inc(xs[c], 16)

        @block.scalar
        def _(scalar):
            # load the Exp table set during the DMA fill
            scalar.memzero(warm)
            scalar.activation(out=warmo, in_=warm, func=AF.Exp)
            for (a0, a1) in spans:
                wait_chunks(scalar, a0, a1)
                scalar.activation(
                    out=esc_all[:, a0:a1, :], in_=xh_all[:, a0:a1, 0, :],
                    func=AF.Exp,
                ).then_inc(act_sem, a1 - a0)

        @block.vector
        def _(vector):
            def rowsum(j):
                vector.wait_ge(act_sem, j + 1)
                ins = vector.scalar_tensor_tensor(
                    out=scr, in0=esc_all[:, j, 0:H1], scalar=1.0,
                    in1=esc_all[:, j, H1:C], op0=OP.mult, op1=OP.add,
                    accum_out=outsb[:, j:j + 1],
                )
                if j % 8 == 7:
                    ins.then_inc(vsum, 1)
                return ins

            for j in range(OUT1):
                rowsum(j)
            # band extraction early: PE is long done by now, and this keeps
            # it off the post-ACT critical path
            vector.wait_ge(pe_tile, NT)
            vector.wait_ge(aux, 16)
            vector.scalar_tensor_tensor(
                out=scrb.rearrange("p (b n) -> p b n", b=NB),
                in0=psum[:, :, 0:P], scalar=1.0,
                in1=band_sb.rearrange("p (b n) -> p b n", b=NB),
                op0=OP.mult, op1=OP.mult,
                accum_out=outsb[:, NT:NT + 1],
            )
            for j in range(OUT1, NT):
                rowsum(j)
            vector.engine_nop().then_inc(vfin, 1)

        @block.tensor
        def _(pe):
            for i in range(NT):
                wait_chunks(pe, i, i + 1)
                for b in range(NB):
                    s0 = BLK[b]
                    mm = pe.matmul(
                        psum[:, b, 0:P], xh_all[:, i, 1, s0:s0 + P],
                        xh_all[:, i, 0, s0:s0 + P],
                        start=(i == 0), stop=(i == NT - 1),
                    )
                mm.then_inc(pe_tile, 1)

    nc.compile()
    return nc


def _shard_inputs(prediction: np.ndarray, target: np.ndarray):
    pred = np.asarray(prediction, dtype=np.float32).reshape(-1, C)
    tgt = np.asarray(target).reshape(-1).astype(np.int64)
    x8 = pred.astype(npf8)
    h8 = np.zeros((ROWS, C), npf8)
    h8[np.arange(ROWS), tgt] = 1.0
    band = _band_masks()
    in_maps = []
    for c in range(NCORES):
        sl = slice(c * RPC, (c + 1) * RPC)
        xh = np.empty((P, NT, 2, C), npf8)
        xh[:, :, 0, :] = x8[sl].reshape(NT, P, C).transpose(1, 0, 2)
        xh[:, :, 1, :] = h8[sl].reshape(NT, P, C).transpose(1, 0, 2)
        in_maps.append({"xh": xh, "band": band})
    return in_maps, tgt


def _host_combine(results, tgt: np.ndarray) -> np.float32:
    w1, w2, w3 = WDEC[1], WDEC[2], WDEC[3]
    t = tgt
    W = (1.0
         + w1 * ((t >= 1).astype(np.float64) + (t <= C - 2))
         + w2 * ((t >= 2).astype(np.float64) + (t <= C - 3))
         + w3 * ((t >= 3).astype(np.float64) + (t <= C - 4)))
    tot = 0.0
    for c, r in enumerate(results):
        o = np.asarray(r["out"], dtype=np.float64)
        S = o[:, 0:NT]                    # S[p, i] = sumexp of row i*P+p
        lse = np.log(S)
        Wc = W[c * RPC:(c + 1) * RPC].reshape(NT, P).T
        tot += (Wc * lse).sum() - o[:, NT].sum()
    return np.float32(tot / ROWS)


def kernel(prediction: np.ndarray, target: np.ndarray, _trace: bool = False):
    nc = _build()
    in_maps, tgt = _shard_inputs(prediction, target)
    res = run_bass_kernel_spmd(
        nc, in_maps, core_ids=list(range(NCORES)), trace=_trace
    )
    loss = _host_combine(res.results, tgt)
    if _trace:
        return loss, res
    return loss
